# revision 4
# baseline (speedup 1.0000x reference)
"""LISTA (learned ISTA) sparse-coding forward pass on 8 Trainium2 NeuronCores.

Problem: I [4,1,192,192] -> im2col(9x9) -> 24 soft-thresholded iterations over
64 filters -> decode -> col2im overlap-add average -> [4,1,192,192].

Sharding: 8 cores = 4 images x 2 position-row halves (92 rows of 184 positions
each). Each core computes its full LISTA pipeline plus the col2im partial sums
for its 100-row output slab; the host merges the 8-row seams between the two
slabs of each image and divides by the overlap counts (pure unshard glue).

Structure (v2):
  - encode: c = WAc @ icol via a col-tiled concurrent MM pair (both position
    halves in one 512-cycle stream); gamma0 = soft(c) from the bf16 copy.
  - loop: block-diag y = Id@c2 + S2@gam2, MMs grouped Id-pass/S-pass per
    PSUM group to amortize LDWEIGHTS; readout split across three engines:
    ACT copies PSUM->bf16, Pool clips, DVE subtracts; one group per
    iteration is done DVE-direct from PSUM to balance engine load.
  - decode: out_all = WW@gam only (row-tiled concurrent pair); the mean
    add-back is NOT decoded per-channel: col2im(mean broadcast) == a 9x9
    backward box filter of the patch means, computed once from the raw slab
    via separable band matmuls and accumulated into the col2im PSUM.
  - col2im: decode chunks stream to a DRAM scratch in row-major order,
    row-block gathers rebuild [row, ch*WO] tiles, 81 shift-band matmuls
    accumulate; host divides by overlap counts.
"""

import contextlib
import numpy as np

# ---------------------------------------------------------------- constants
B, H, Wimg = 4, 192, 192
K = 9
F = 64
NCH = K * K  # 81
HO = H - K + 1  # 184
WO = Wimg - K + 1  # 184
UNF = 24
N_CORES = 8

ROWS = HO // 2  # 92 position rows per core
SLAB = ROWS + K - 1  # 100 image/output rows per core
NPOS = ROWS * WO  # 16928 positions per core
HALFR = ROWS // 2  # 46 rows per block-diag half
HALF = HALFR * WO  # 8464 columns per half

# loop grouping: 5 x 1536 + 784 (PSUM: 3+3+2 banks)
GBIG = 1536
GROUPS = [(i * GBIG, (i + 1) * GBIG) for i in range(5)] + [(5 * GBIG, HALF)]
MMN = 512

# decode grouping: 5 position rows per group
DECR = 5
DCH = DECR * WO  # 920

# weight blob layout: (name, partitions, cols) — bf16
BLOB_SPEC = [
    ("wact", NCH, F), ("sbd", 128, 128), ("id128", 128, 128),
    ("wwb", 128, NCH), ("eshb", ROWS, K * SLAB),
    ("bandf", SLAB, ROWS), ("bandb", ROWS, SLAB),
]
BLOBC = sum(nf for _, _, nf in BLOB_SPEC)

_STATE = {}


def _split_multi_waits(nc, mybir):
    """This walrus build supports a single sync-wait slot per instruction.
    Move extra waits onto preceding same-engine no-ops (same semantics:
    program order on one engine; all waits clear before the instruction)."""
    cnt = 0
    for fn in nc.m.functions:
        for bb in fn.blocks:
            insts = bb.instructions
            need = False
            for ins in insts:
                si = ins.sync_info
                if si is not None and si.on_wait is not None and len(si.on_wait) > 1:
                    need = True
                    break
            if not need:
                continue
            out = []
            for ins in insts:
                si = ins.sync_info
                if si is not None and si.on_wait is not None and len(si.on_wait) > 1:
                    waits = list(si.on_wait)
                    for w in waits[:-1]:
                        cnt += 1
                        nop = mybir.InstNoOp(name=f"wsplit-{cnt}", ins=[], outs=[])
                        nop.engine = ins.engine
                        nop.sync_info = mybir.SyncInfo(on_wait=[w], on_update=[])
                        out.append(nop)
                    ins.sync_info = mybir.SyncInfo(
                        on_wait=[waits[-1]], on_update=list(si.on_update or [])
                    )
                out.append(ins)
            bb.instructions = out
    return cnt


def _build():
    import concourse.bass as bass
    import concourse.mybir as mybir
    import concourse.tile as tile

    f32 = mybir.dt.float32
    bf16 = mybir.dt.bfloat16
    Alu = mybir.AluOpType

    nc = bass.Bass("TRN2", target_bir_lowering=False, debug=False)

    imgw = nc.dram_tensor("imgw", [K * SLAB * WO], bf16, kind="ExternalInput").ap()
    blob_d = nc.dram_tensor("blob", [128, BLOBC], bf16, kind="ExternalInput").ap()
    lams_d = nc.dram_tensor("lams", [128, 2], f32, kind="ExternalInput").ap()
    out_d = nc.dram_tensor("out", [SLAB, Wimg], f32, kind="ExternalOutput").ap()
    obuf = nc.dram_tensor("obuf", [ROWS * NCH * WO], bf16, kind="Internal").ap()

    with tile.TileContext(nc) as tc:
        with contextlib.ExitStack() as ctx:
            wpool = ctx.enter_context(tc.tile_pool(name="w", bufs=1))
            big = ctx.enter_context(tc.tile_pool(name="big", bufs=1))
            pp = ctx.enter_context(tc.tile_pool(name="ps", bufs=2, space="PSUM"))
            pt = ctx.enter_context(tc.tile_pool(name="pt", bufs=1, space="PSUM"))
            sc = ctx.enter_context(tc.tile_pool(name="sc", bufs=2))
            ring = ctx.enter_context(tc.tile_pool(name="ring", bufs=3))

            blob = wpool.tile([128, BLOBC], bf16)
            nc.sync.dma_start(blob[:], blob_d)
            o = {}
            col = 0
            for name, np_, nf in BLOB_SPEC:
                o[name] = (np_, col, nf)
                col += nf
            def bl(name):
                np_, c0, nf = o[name]
                return blob[0:np_, c0:c0 + nf]
            wact = bl("wact"); sbd = bl("sbd"); id128 = bl("id128")
            wwb = bl("wwb"); eshb = bl("eshb")
            bandf = bl("bandf"); bandb = bl("bandb")
            lams = wpool.tile([128, 2], f32)
            nc.sync.dma_start(lams[:], lams_d)
            lam = lams[:, 0:1]
            nlam = lams[:, 1:2]

            icol = big.tile([NCH, NPOS], bf16, tag="icol")
            c2 = big.tile([128, HALF], bf16)
            gam = big.tile([128, HALF], bf16)
            slabt = wpool.tile([SLAB, 192], bf16)
            rs_sb = wpool.tile([SLAB, HO], bf16)
            mean_sb = wpool.tile([ROWS, HO], bf16)
            rb_sb = wpool.tile([ROWS, 192], bf16)
            acc = wpool.tile([SLAB, Wimg], f32)

            # ---- input DMAs: slab pieces for the mean path + im2col gathers.
            # imgw[kw] = slab[:, kw:kw+WO]; channel (kh, kw) of icol is
            # imgw[kw][kh:kh+ROWS] — one contiguous 3D gather per quarter.
            nc.sync.dma_start(slabt[:, 0:WO],
                              bass.AP(imgw.tensor, 0, [[WO, SLAB], [1, WO]]))
            nc.sync.dma_start(slabt[:, WO:192],
                              bass.AP(imgw.tensor, (K - 1) * SLAB * WO + WO - 8,
                                      [[WO, SLAB], [1, 8]]))
            for hh in range(2):
                eng = nc.sync if hh == 0 else nc.gpsimd
                for q in range(2):
                    r0 = hh * HALFR + q * (HALFR // 2)
                    r1 = hh * HALFR + (q + 1) * (HALFR // 2)
                    eng.dma_start(
                        icol[:, r0 * WO:r1 * WO],
                        bass.AP(imgw.tensor, r0 * WO,
                                [[WO, K], [SLAB * WO, K], [1, (r1 - r0) * WO]]))

            # ---- mean path (pre-loop): bm = backward-box9(mean), mean =
            # forward-box9(slab)/81, separable via band matmuls.
            ps1 = pt.tile([128, 784], f32, tag="pt")
            id100 = id128[0:SLAB, 0:SLAB]
            for kw in range(K):
                nc.tensor.matmul(ps1[0:SLAB, 0:HO], id100, slabt[:, kw:kw + HO],
                                 start=(kw == 0), stop=(kw == K - 1))
            nc.vector.tensor_copy(rs_sb[:], ps1[0:SLAB, 0:HO])
            ps2 = pt.tile([128, 784], f32, tag="pt")
            nc.tensor.matmul(ps2[0:ROWS, 0:HO], bandf, rs_sb[:],
                             start=True, stop=True)
            nc.vector.tensor_copy(mean_sb[:], ps2[0:ROWS, 0:HO])
            ps3 = pt.tile([128, 784], f32, tag="pt")
            id92 = id128[0:ROWS, 0:ROWS]
            for kw in range(K):
                nc.tensor.matmul(ps3[0:ROWS, kw:kw + HO], id92, mean_sb[:],
                                 start=(kw == 0), stop=(kw == K - 1))
            nc.vector.tensor_copy(rb_sb[:], ps3[0:ROWS, 0:192])

            # ---- encode: c = WAc @ icol, col-tiled pair (h1 -> psum[0:64],
            # h2 -> psum[64:128]); copy once to bf16 c2, gamma0 = soft(c2).
            for gi, (g0, g1) in enumerate(GROUPS):
                gn = g1 - g0
                ps = (pp if gn == GBIG else pt).tile(
                    [128, gn if gn == GBIG else 784], f32,
                    tag="ps" if gn == GBIG else "pt")
                for c0 in range(0, gn, MMN):
                    n = min(MMN, gn - c0)
                    nc.tensor.matmul(ps[0:F, c0:c0 + n], wact,
                                     icol[:, g0 + c0:g0 + c0 + n],
                                     start=True, stop=True)
                    nc.tensor.matmul(ps[F:128, c0:c0 + n], wact,
                                     icol[:, HALF + g0 + c0:HALF + g0 + c0 + n],
                                     start=True, stop=True)
                nc.scalar.copy(c2[:, g0:g1], ps[:, 0:gn])
                z = sc.tile([128, GBIG], bf16, tag="z")
                nc.gpsimd.tensor_scalar(z[:, 0:gn], c2[:, g0:g1],
                                        lam, nlam, Alu.min, Alu.max)
                nc.vector.tensor_tensor(gam[:, g0:g1], c2[:, g0:g1],
                                        z[:, 0:gn], Alu.subtract)

            # ---- 23 iterations: y = Id@c2 + S2@gam; gam' = y - clip(y).
            # Groups paired so LDWEIGHTS amortizes (Id-pass then S-pass over
            # the pair); readout: groups 0-3+tail via ACT-copy/Pool-clip/
            # DVE-sub, group 4 DVE-direct from PSUM.
            for _t in range(UNF - 1):
                for p0 in range(0, len(GROUPS), 2):
                    pair = GROUPS[p0:p0 + 2]
                    tiles = []
                    for (g0, g1) in pair:
                        gn = g1 - g0
                        ps = (pp if gn == GBIG else pt).tile(
                            [128, gn if gn == GBIG else 784], f32,
                            tag="ps" if gn == GBIG else "pt")
                        tiles.append(ps)
                    for (g0, g1), ps in zip(pair, tiles):
                        for c0 in range(0, g1 - g0, MMN):
                            n = min(MMN, g1 - g0 - c0)
                            nc.tensor.matmul(ps[:, c0:c0 + n], id128,
                                             c2[:, g0 + c0:g0 + c0 + n],
                                             start=True, stop=False)
                    for (g0, g1), ps in zip(pair, tiles):
                        for c0 in range(0, g1 - g0, MMN):
                            n = min(MMN, g1 - g0 - c0)
                            nc.tensor.matmul(ps[:, c0:c0 + n], sbd,
                                             gam[:, g0 + c0:g0 + c0 + n],
                                             start=False, stop=True)
                    for gi, ((g0, g1), ps) in enumerate(zip(pair, tiles)):
                        gn = g1 - g0
                        if p0 + gi == 4:
                            # DVE-direct from PSUM
                            z = sc.tile([128, GBIG], bf16, tag="z")
                            nc.vector.tensor_scalar(z[:, 0:gn], ps[:, 0:gn],
                                                    lam, nlam, Alu.min, Alu.max)
                            nc.vector.tensor_tensor(gam[:, g0:g1], ps[:, 0:gn],
                                                    z[:, 0:gn], Alu.subtract)
                        else:
                            yb = sc.tile([128, GBIG], bf16, tag="yb")
                            nc.scalar.copy(yb[:, 0:gn], ps[:, 0:gn])
                            z = sc.tile([128, GBIG], bf16, tag="z")
                            if p0 + gi == 5:
                                nc.vector.tensor_scalar(
                                    z[:, 0:gn], yb[:, 0:gn],
                                    lam, nlam, Alu.min, Alu.max)
                            else:
                                nc.gpsimd.tensor_scalar(
                                    z[:, 0:gn], yb[:, 0:gn],
                                    lam, nlam, Alu.min, Alu.max)
                            nc.vector.tensor_tensor(gam[:, g0:g1], yb[:, 0:gn],
                                                    z[:, 0:gn], Alu.subtract)

            # ---- decode: out_all = WW@gam, row-tiled concurrent MM pairs;
            # psum -> bf16 ring -> DRAM obuf (row-major [row][ch][x]); obuf
            # writes split at position-row boundaries (<=3 DMAs per group).
            ww1 = wwb[0:F, :]
            ww2 = wwb[F:128, :]
            di = 0
            for half in range(2):
                gsl = gam[0:F, :] if half == 0 else gam[F:128, :]
                ww = ww1 if half == 0 else ww2
                for (g0, g1) in GROUPS:
                    gn = g1 - g0
                    ps = (pp if gn == GBIG else pt).tile(
                        [128, gn if gn == GBIG else 784], f32,
                        tag="ps" if gn == GBIG else "pt")
                    for cc in range(0, gn, MMN):
                        n = min(MMN, gn - cc)
                        nc.tensor.matmul(ps[0:NCH, cc:cc + n], ww,
                                         gsl[:, g0 + cc:g0 + cc + n],
                                         start=True, stop=True)
                    rg = ring.tile([NCH, GBIG], bf16, tag="ring")
                    if di % 2 == 0:
                        nc.scalar.copy(rg[:, 0:gn], ps[0:NCH, 0:gn])
                    else:
                        nc.vector.tensor_copy(rg[:, 0:gn], ps[0:NCH, 0:gn])
                    di += 1
                    # obuf writes, split at row boundaries
                    rbase = half * HALFR
                    p = g0
                    while p < g1:
                        r = p // WO
                        pe = min(g1, (r + 1) * WO)
                        if p == r * WO and pe - p == WO:
                            # run of full rows
                            nfull = (g1 - p) // WO
                            pe = p + nfull * WO
                            dst = bass.AP(obuf.tensor, (rbase + r) * NCH * WO,
                                          [[WO, NCH], [NCH * WO, nfull],
                                           [1, WO]])
                        else:
                            dst = bass.AP(obuf.tensor,
                                          (rbase + r) * NCH * WO + (p - r * WO),
                                          [[WO, NCH], [1, pe - p]])
                        eng = nc.sync if di % 2 == 0 else nc.gpsimd
                        eng.dma_start(dst, rg[:, p - g0:pe - g0])
                        p = pe

            # ---- col2im: row-block gathers (contiguous per row) + 81
            # shift-band matmuls + the bm mean term, all into one PSUM.
            stall = big.tile([ROWS, NCH * WO], bf16, tag="icol")
            NBLK = 4
            for b_ in range(NBLK):
                rr0 = b_ * (ROWS // NBLK)
                rr1 = (b_ + 1) * (ROWS // NBLK)
                eng = (nc.sync, nc.gpsimd, nc.sync, nc.gpsimd)[b_ % 4]
                eng.dma_start(
                    stall[rr0:rr1, :],
                    bass.AP(obuf.tensor, rr0 * NCH * WO,
                            [[NCH * WO, rr1 - rr0], [1, NCH * WO]]))
            ops = pp.tile([128, GBIG], f32, tag="ps")
            for kh in range(K):
                lhs = eshb[:, kh * SLAB:(kh + 1) * SLAB]
                for kw in range(K):
                    nc.tensor.matmul(
                        ops[0:SLAB, kw:kw + WO], lhs,
                        stall[:, (kh * K + kw) * WO:(kh * K + kw + 1) * WO],
                        start=(kh == 0 and kw == 0), stop=False)
            nc.tensor.matmul(ops[0:SLAB, 0:192], bandb, rb_sb[:],
                             start=False, stop=True)
            nc.scalar.copy(acc[:], ops[0:SLAB, 0:Wimg])
            nc.sync.dma_start(out_d, acc[:])

    import concourse.mybir as mybir
    _split_multi_waits(nc, mybir)
    return nc


def _get_nc():
    if "nc" not in _STATE:
        _STATE["nc"] = _build()
    return _STATE["nc"]


def _make_in_maps(I, WA, WD, WW, lmbda):
    import ml_dtypes  # noqa: F401
    I = np.ascontiguousarray(np.asarray(I, np.float32))
    WA = np.asarray(WA, np.float32)
    WD = np.asarray(WD, np.float32)
    WW = np.asarray(WW, np.float32)
    lam = np.asarray(lmbda, np.float32).reshape(F)
    assert I.shape == (B, 1, H, Wimg)

    WAc = (WA - WA.mean(axis=1, keepdims=True)).astype(np.float32)  # [64,81]
    S = (np.eye(F, dtype=np.float32) - WA @ WD).astype(np.float32)  # [64,64]
    sbd = np.zeros((128, 128), np.float32)
    sbd[0:F, 0:F] = S.T
    sbd[F:128, F:128] = S.T
    id128 = np.eye(128, dtype=np.float32)
    wwb = np.zeros((128, NCH), np.float32)
    wwb[0:F, :] = WW.T
    wwb[F:128, :] = WW.T
    lam128 = np.concatenate([lam, lam]).reshape(128, 1).astype(np.float32)
    esh = np.zeros((ROWS, K * SLAB), np.float32)  # lhsT per kh: E[r,y]=1 iff y=r+kh
    for kh in range(K):
        for rr in range(ROWS):
            esh[rr, kh * SLAB + rr + kh] = 1.0
    bandf = np.zeros((SLAB, ROWS), np.float32)
    for rr in range(SLAB):
        for r in range(ROWS):
            if r <= rr <= r + K - 1:
                bandf[rr, r] = 1.0 / NCH
    bandb = np.zeros((ROWS, SLAB), np.float32)
    for r in range(ROWS):
        for y in range(SLAB):
            if 0 <= y - r <= K - 1:
                bandb[r, y] = 1.0
    vals = {"wact": WAc.T, "sbd": sbd, "id128": id128, "wwb": wwb,
            "eshb": esh, "bandf": bandf, "bandb": bandb}
    blob = np.zeros((128, BLOBC), np.float32)
    col = 0
    for name, np_, nf in BLOB_SPEC:
        v = np.asarray(vals[name], np.float32)
        assert v.shape == (np_, nf), (name, v.shape)
        blob[0:np_, col:col + nf] = v
        col += nf
    lams = np.concatenate([lam128, -lam128], axis=1).astype(np.float32)

    shared = {"blob": blob.astype(ml_dtypes.bfloat16), "lams": lams}
    in_maps = []
    for core in range(N_CORES):
        b, h = core // 2, core % 2
        r0 = h * ROWS
        slab = I[b, 0, r0:r0 + SLAB, :]
        imgw = np.stack([slab[:, kw:kw + WO] for kw in range(K)], axis=0)
        in_maps.append({"imgw": np.ascontiguousarray(imgw).reshape(-1).astype(
            ml_dtypes.bfloat16), **shared})
    return in_maps


def _unshard(results):
    cnt = np.zeros((H, Wimg), np.float32)
    for kh in range(K):
        for kw in range(K):
            cnt[kh:kh + HO, kw:kw + WO] += 1.0
    out = np.zeros((B, 1, H, Wimg), np.float32)
    for b in range(B):
        acc = np.zeros((H, Wimg), np.float32)
        acc[0:SLAB, :] += results[2 * b]["out"]
        acc[ROWS:ROWS + SLAB, :] += results[2 * b + 1]["out"]
        out[b, 0] = acc / cnt
    return out


def kernel(I, WA, WD, WW, lmbda, kernel_size=9, stride=1, unfoldings=24, **_kw):
    from concourse import bass_utils

    assert int(kernel_size) == K and int(stride) == 1 and int(unfoldings) == UNF
    in_maps = _make_in_maps(I, WA, WD, WW, lmbda)
    nc = _get_nc()
    last = None
    for _attempt in range(3):
        try:
            res = bass_utils.run_bass_kernel_spmd(
                nc, in_maps, core_ids=list(range(N_CORES)))
            return _unshard(res.results)
        except Exception as e:  # transient NRT device errors: retry
            last = e
    raise last
